# revision 2
# baseline (speedup 1.0000x reference)
"""AttentionPooling (segment softmax + weighted segment sum) on 8 trn2 cores.

Math (per graph g): out[g] = sum_n softmax_g(s)_n * x[n] over nodes n with
batch[n] == g, where s = tanh(x @ W1 + b1) @ W2 + b2.

Design (measured on hw, fastest of the variants tried):
  * batch is sorted -> shard by graph (128 contiguous graphs per core): pure
    data parallel, no collectives; host gathers the 8 [128, 256] outputs.
  * |s| <= ||W2||_1 + |b2| ~ 11.3, so exp never overflows fp32 -> accumulate
    unnormalized exp(s)*[x|1] into one PSUM tile and divide once at the end.
  * Two HBM streams: xaug = [x|1] in bf16 with nodes on partitions (pooling
    rhs), and xT in fp8-e4m3 with features on partitions (attention-MLP rhs).
    fp8 on the scores path costs ~1.3e-2 rel err (gate 2e-2) and cuts both
    HBM traffic and PE work.
  * MLP layer 1 runs as a single fp8 DoubleRow matmul per 512-node chunk
    (both K-tiles in one pass, 0.5 cyc/row).
  * The segment-sum is a TensorE matmul with a weighted one-hot matrix
    st[n, g] = exp(s_n) * (batch[n] == g), 128 local graphs == PSUM
    partitions; the one-hot is built on DVE (is_equal+mult tensor_scalar;
    GPSIMD is ~10x slower than its cost model for this op - measured).
  * Software pipeline: per chunk i the stages M(i) = MLP matmuls + tanh,
    S(i-1) = score matmuls + exp + one-hot, D(i-2) = pooling accumulation
    run in one PE pass, so PE never waits on ScalarE/DVE results of the
    same chunk (cross-engine dependencies get a full chunk of slack).

The `repeats` parameter wraps the whole pass in a hardware For_i loop; each
rep redoes the accumulation from scratch (output = last pass, still correct)
- used by test.py to measure per-pass HW time by wall-clock slope.
"""

import sys

from contextlib import ExitStack

import numpy as np

for _p in ("/opt/trn_rl_repo",):
    if _p not in sys.path:
        sys.path.insert(0, _p)

import ml_dtypes

import concourse.bass as bass
import concourse.bacc as bacc
import concourse.tile as tile
from concourse import mybir

N_NODES = 500_000
HIDDEN = 256
NUM_GRAPHS = 1024
N_CORES = 8
G_LOC = NUM_GRAPHS // N_CORES  # 128 graphs per core == PSUM partition dim
H = HIDDEN // 2  # 128 hidden units in the attention MLP
BLK = 128  # nodes per block (pool matmul contraction tile)
NBPC = 4  # blocks per chunk
CH = BLK * NBPC  # 512 nodes per compute chunk (one PSUM bank at fp32)
CPS = 4  # chunks per DMA super-chunk
SUP = CH * CPS  # 2048 nodes per DMA super-chunk
NB = NBPC * CPS  # 16 blocks per super-chunk
BF16 = mybir.dt.bfloat16
FP8 = mybir.dt.float8e4
F32 = mybir.dt.float32

_PROGRAM_CACHE: dict[int, bass.Bass] = {}


def build_program(n_pad: int, repeats: int = 1) -> bass.Bass:
    assert n_pad % SUP == 0
    nblk = n_pad // BLK
    nsup = n_pad // SUP

    nc = bacc.Bacc("TRN2")
    # host-swizzled so each super-chunk DMA reads one contiguous run per
    # partition: xaug[s, p, b, f] = [x | 1.0][s*SUP + b*BLK + p, f]
    xaug = nc.dram_tensor(
        "xaug", [nsup, BLK, NB, HIDDEN + 1], BF16, kind="ExternalInput"
    )
    # xT[s, p, j, n] = x[s*SUP + n, BLK*j + p] in fp8-e4m3
    xT = nc.dram_tensor("xT", [nsup, BLK, 2, SUP], FP8, kind="ExternalInput")
    bcols = nc.dram_tensor("bcols", [BLK, nblk], F32, kind="ExternalInput")
    w1 = nc.dram_tensor("w1", [HIDDEN, H], FP8, kind="ExternalInput")
    w2 = nc.dram_tensor("w2", [H, 1], BF16, kind="ExternalInput")
    b1 = nc.dram_tensor("b1", [H, 1], F32, kind="ExternalInput")
    b2 = nc.dram_tensor("b2", [BLK, 1], F32, kind="ExternalInput")
    out = nc.dram_tensor("out", [G_LOC, HIDDEN], F32, kind="ExternalOutput")

    with tile.TileContext(nc) as tc, ExitStack() as ctx:
        singles = ctx.enter_context(tc.tile_pool(name="singles", bufs=1))
        xa_pool = ctx.enter_context(tc.tile_pool(name="xa", bufs=3))
        xt_pool = ctx.enter_context(tc.tile_pool(name="xt", bufs=3))
        tt_pool = ctx.enter_context(tc.tile_pool(name="tt", bufs=4))
        st_pool = ctx.enter_context(tc.tile_pool(name="st", bufs=12))
        e_pool = ctx.enter_context(tc.tile_pool(name="e", bufs=4))
        hp_pool = ctx.enter_context(tc.tile_pool(name="hp", bufs=2, space="PSUM"))
        sp_pool = ctx.enter_context(tc.tile_pool(name="sp", bufs=2, space="PSUM"))
        acc_pool = ctx.enter_context(tc.tile_pool(name="acc", bufs=1, space="PSUM"))

        w1_sb = singles.tile([BLK, 2, H], FP8)
        nc.sync.dma_start(out=w1_sb[:, 0, :], in_=w1[0:BLK, :])
        nc.sync.dma_start(out=w1_sb[:, 1, :], in_=w1[BLK : 2 * BLK, :])
        w2_sb = singles.tile([H, 1], BF16)
        nc.sync.dma_start(out=w2_sb, in_=w2[:, :])
        b1_sb = singles.tile([H, 1], F32)
        nc.sync.dma_start(out=b1_sb, in_=b1[:, :])
        b2_sb = singles.tile([BLK, 1], F32)
        nc.sync.dma_start(out=b2_sb, in_=b2[:, :])
        bc_sb = singles.tile([BLK, nblk], F32)
        nc.sync.dma_start(out=bc_sb, in_=bcols[:, :])
        # bf16 iota: values 0..127 exact; 2-byte operands enable DVE 2x mode
        iota_sb = singles.tile([BLK, G_LOC], BF16)
        nc.gpsimd.iota(
            out=iota_sb,
            pattern=[[1, G_LOC]],
            base=0,
            channel_multiplier=0,
            allow_small_or_imprecise_dtypes=True,
        )

        acc = acc_pool.tile([G_LOC, HIDDEN + 1], F32)

        def body():
            NCH = nsup * CPS
            chunks = [(s, q) for s in range(nsup) for q in range(CPS)]
            state = [dict() for _ in range(NCH)]
            supers = {}

            def dma_super(s):
                xa = xa_pool.tile([BLK, NB, HIDDEN + 1], BF16)
                nc.sync.dma_start(out=xa, in_=xaug[s])
                xt = xt_pool.tile([BLK, 2, SUP], FP8)
                nc.sync.dma_start(out=xt, in_=xT[s])
                supers[s] = {"xa": xa, "xt": xt}

            def stage_M(i):
                s, q = chunks[i]
                xt = supers[s]["xt"]
                hp = hp_pool.tile([H, CH], F32)
                nc.tensor.matmul(
                    hp,
                    lhsT=w1_sb,
                    rhs=xt[:, :, q * CH : (q + 1) * CH],
                    perf_mode=mybir.MatmulPerfMode.DoubleRow,
                    start=True,
                    stop=True,
                )
                tt = tt_pool.tile([H, CH], BF16)
                nc.scalar.activation(
                    out=tt, in_=hp, func=mybir.ActivationFunctionType.Tanh, bias=b1_sb
                )
                state[i]["tt"] = tt

            def stage_S(i):
                s, q = chunks[i]
                tt = state[i]["tt"]
                sp = sp_pool.tile([BLK, NBPC], F32)
                for b in range(NBPC):
                    nc.tensor.matmul(
                        sp[:, b : b + 1],
                        lhsT=tt[:, b * BLK : (b + 1) * BLK],
                        rhs=w2_sb,
                        start=True,
                        stop=True,
                    )
                ee = e_pool.tile([BLK, NBPC], F32)
                nc.scalar.activation(
                    out=ee, in_=sp, func=mybir.ActivationFunctionType.Exp, bias=b2_sb
                )
                sts = []
                for b in range(NBPC):
                    st = st_pool.tile([BLK, G_LOC], BF16, tag="st")
                    j = (s * CPS + q) * NBPC + b
                    nc.vector.tensor_scalar(
                        out=st,
                        in0=iota_sb,
                        scalar1=bc_sb[:, j : j + 1],
                        scalar2=ee[:, b : b + 1],
                        op0=mybir.AluOpType.is_equal,
                        op1=mybir.AluOpType.mult,
                    )
                    sts.append(st)
                state[i]["sts"] = sts

            def stage_D(i):
                s, q = chunks[i]
                xa = supers[s]["xa"]
                sts = state[i]["sts"]
                for b in range(NBPC):
                    nc.tensor.matmul(
                        acc,
                        lhsT=sts[b],
                        rhs=xa[:, q * NBPC + b, :],
                        start=(i == 0 and b == 0),
                        stop=(i == NCH - 1 and b == NBPC - 1),
                    )

            dma_super(0)
            for i in range(NCH + 2):
                if i < NCH:
                    s, q = chunks[i]
                    if q == 0 and s + 1 < nsup:
                        dma_super(s + 1)
                    stage_M(i)
                if 0 <= i - 1 < NCH:
                    stage_S(i - 1)
                if 0 <= i - 2 < NCH:
                    stage_D(i - 2)

        if repeats == 1:
            body()
        else:
            with tc.For_i(0, repeats):
                body()

        denom = singles.tile([G_LOC, 1], F32)
        nc.vector.tensor_scalar_max(
            out=denom, in0=acc[:, HIDDEN : HIDDEN + 1], scalar1=1e-30
        )
        rdenom = singles.tile([G_LOC, 1], F32)
        nc.vector.reciprocal(out=rdenom, in_=denom)
        out_sb = singles.tile([G_LOC, HIDDEN], F32)
        nc.vector.tensor_scalar_mul(out=out_sb, in0=acc[:, 0:HIDDEN], scalar1=rdenom)
        nc.sync.dma_start(out=out[:, :], in_=out_sb)

    nc.finalize()
    return nc


def make_in_maps(x, batch, W1, b1, W2, b2):
    """Shard by graph (128 contiguous graphs per core), pad node counts to a
    common multiple of SUP, and lay out the per-core device arrays."""
    x = np.asarray(x, dtype=np.float32)
    batch = np.asarray(batch)
    bounds = np.searchsorted(batch, np.arange(0, NUM_GRAPHS + 1, G_LOC))
    n_loc_max = int(np.diff(bounds).max())
    n_pad = max(SUP, ((n_loc_max + SUP - 1) // SUP) * SUP)

    w1_q = np.asarray(W1, np.float32).astype(ml_dtypes.float8_e4m3)
    w2_bf = np.asarray(W2, np.float32).reshape(H, 1).astype(ml_dtypes.bfloat16)
    b1_f = np.asarray(b1, np.float32).reshape(H, 1)
    b2_f = np.full((BLK, 1), np.float32(np.asarray(b2).reshape(-1)[0]), np.float32)

    in_maps = []
    for c in range(N_CORES):
        s, e = int(bounds[c]), int(bounds[c + 1])
        nloc = e - s
        xs = x[s:e]
        nsup = n_pad // SUP
        xa = np.zeros((n_pad, HIDDEN + 1), ml_dtypes.bfloat16)
        xa[:nloc, :HIDDEN] = xs
        xa[:nloc, HIDDEN] = 1.0
        # [s*SUP + b*BLK + p, f] -> [s, p, b, f]
        xa = np.ascontiguousarray(
            xa.reshape(nsup, NB, BLK, HIDDEN + 1).transpose(0, 2, 1, 3)
        )
        # [s, p, j, n] = x[s*SUP + n, BLK*j + p]
        xT = np.zeros((HIDDEN, n_pad), ml_dtypes.float8_e4m3)
        xT[:, :nloc] = xs.T.astype(ml_dtypes.float8_e4m3)
        xT = np.ascontiguousarray(xT.reshape(2, BLK, nsup, SUP).transpose(2, 1, 0, 3))
        bl = np.full((n_pad,), -1.0, np.float32)
        bl[:nloc] = batch[s:e].astype(np.float32) - np.float32(c * G_LOC)
        bcols = np.ascontiguousarray(bl.reshape(n_pad // BLK, BLK).T)
        in_maps.append(
            {
                "xaug": xa,
                "xT": xT,
                "bcols": bcols,
                "w1": w1_q,
                "w2": w2_bf,
                "b1": b1_f,
                "b2": b2_f,
            }
        )
    return in_maps, n_pad


def kernel(x, batch, W1, b1, W2, b2):
    from concourse.bass_utils import run_bass_kernel_spmd

    in_maps, n_pad = make_in_maps(x, batch, W1, b1, W2, b2)
    nc = _PROGRAM_CACHE.get(n_pad)
    if nc is None:
        nc = build_program(n_pad)
        _PROGRAM_CACHE[n_pad] = nc
    res = run_bass_kernel_spmd(nc, in_maps, list(range(N_CORES)))
    return np.concatenate([res.results[c]["out"] for c in range(N_CORES)], axis=0)


# revision 3
# speedup vs baseline: 1.0974x; 1.0974x over previous
"""AttentionPooling (segment softmax + weighted segment sum) on 8 trn2 cores.

Math (per graph g): out[g] = sum_n softmax_g(s)_n * x[n] over nodes n with
batch[n] == g, where s = tanh(x @ W1 + b1) @ W2 + b2.

Design (measured on hw, fastest of the variants tried):
  * batch is sorted -> shard by graph (128 contiguous graphs per core): pure
    data parallel, no collectives; host gathers the 8 [128, 256] outputs.
  * |s| <= ||W2||_1 + |b2| ~ 11.3, so exp never overflows fp32 -> accumulate
    unnormalized exp(s)*[x|1] into one PSUM tile and divide once at the end.
  * Two HBM streams: xaug = [x|1] in bf16 with nodes on partitions (pooling
    rhs), and xT in fp8-e4m3 with features on partitions (attention-MLP rhs).
    fp8 on the scores path costs ~1.3e-2 rel err (gate 2e-2) and cuts both
    HBM traffic and PE work.
  * MLP layer 1 runs as a single fp8 DoubleRow matmul per 512-node chunk
    (both K-tiles in one pass, 0.5 cyc/row).
  * The segment-sum is a TensorE matmul with a weighted one-hot matrix
    st[n, g] = exp(s_n) * (batch[n] == g), 128 local graphs == PSUM
    partitions; the one-hot is built on DVE (is_equal+mult tensor_scalar;
    GPSIMD is ~10x slower than its cost model for this op - measured).
  * Software pipeline: per chunk i the stages M(i) = MLP matmuls + tanh,
    S(i-1) = score matmuls + exp + one-hot, D(i-2) = pooling accumulation
    run in one PE pass, so PE never waits on ScalarE/DVE results of the
    same chunk (cross-engine dependencies get a full chunk of slack).

The `repeats` parameter wraps the whole pass in a hardware For_i loop; each
rep redoes the accumulation from scratch (output = last pass, still correct)
- used by test.py to measure per-pass HW time by wall-clock slope.
"""

import sys

from contextlib import ExitStack

import numpy as np

for _p in ("/opt/trn_rl_repo",):
    if _p not in sys.path:
        sys.path.insert(0, _p)

import ml_dtypes

import concourse.bass as bass
import concourse.bacc as bacc
import concourse.tile as tile
from concourse import mybir

N_NODES = 500_000
HIDDEN = 256
NUM_GRAPHS = 1024
N_CORES = 8
G_LOC = NUM_GRAPHS // N_CORES  # 128 graphs per core == PSUM partition dim
H = HIDDEN // 2  # 128 hidden units in the attention MLP
BLK = 128  # nodes per block (pool matmul contraction tile)
NBPC = 4  # blocks per chunk
CH = BLK * NBPC  # 512 nodes per compute chunk (one PSUM bank at fp32)
CPS = 4  # chunks per DMA super-chunk
SUP = CH * CPS  # 2048 nodes per DMA super-chunk
NB = NBPC * CPS  # 16 blocks per super-chunk
BF16 = mybir.dt.bfloat16
FP8 = mybir.dt.float8e4
F32 = mybir.dt.float32

_PROGRAM_CACHE: dict[int, bass.Bass] = {}


def build_program(n_pad: int, repeats: int = 1) -> bass.Bass:
    assert n_pad % SUP == 0
    nblk = n_pad // BLK
    nsup = n_pad // SUP

    nc = bacc.Bacc("TRN2")
    # host-swizzled so each super-chunk DMA reads one contiguous run per
    # partition: xaug[s, p, b, f] = [x | 1.0][s*SUP + b*BLK + p, f]
    xaug = nc.dram_tensor(
        "xaug", [nsup, BLK, NB, HIDDEN + 1], BF16, kind="ExternalInput"
    )
    # xT[s, p, j, n] = x[s*SUP + n, BLK*j + p] in fp8-e4m3
    xT = nc.dram_tensor("xT", [nsup, BLK, 2, SUP], FP8, kind="ExternalInput")
    bcols = nc.dram_tensor("bcols", [BLK, nblk], F32, kind="ExternalInput")
    w1 = nc.dram_tensor("w1", [HIDDEN, H], FP8, kind="ExternalInput")
    w2 = nc.dram_tensor("w2", [H, 1], BF16, kind="ExternalInput")
    b1 = nc.dram_tensor("b1", [H, 1], F32, kind="ExternalInput")
    b2 = nc.dram_tensor("b2", [BLK, 1], F32, kind="ExternalInput")
    out = nc.dram_tensor("out", [G_LOC, HIDDEN], F32, kind="ExternalOutput")

    with tile.TileContext(nc) as tc, ExitStack() as ctx:
        singles = ctx.enter_context(tc.tile_pool(name="singles", bufs=1))
        xa_pool = ctx.enter_context(tc.tile_pool(name="xa", bufs=3))
        xt_pool = ctx.enter_context(tc.tile_pool(name="xt", bufs=3))
        tt_pool = ctx.enter_context(tc.tile_pool(name="tt", bufs=4))
        st_pool = ctx.enter_context(tc.tile_pool(name="st", bufs=12))
        e_pool = ctx.enter_context(tc.tile_pool(name="e", bufs=4))
        hp_pool = ctx.enter_context(tc.tile_pool(name="hp", bufs=2, space="PSUM"))
        sp_pool = ctx.enter_context(tc.tile_pool(name="sp", bufs=2, space="PSUM"))
        acc_pool = ctx.enter_context(tc.tile_pool(name="acc", bufs=1, space="PSUM"))

        w1_sb = singles.tile([BLK, 2, H], FP8)
        nc.sync.dma_start(out=w1_sb[:, 0, :], in_=w1[0:BLK, :])
        nc.sync.dma_start(out=w1_sb[:, 1, :], in_=w1[BLK : 2 * BLK, :])
        w2_sb = singles.tile([H, 1], BF16)
        nc.sync.dma_start(out=w2_sb, in_=w2[:, :])
        b1_sb = singles.tile([H, 1], F32)
        nc.sync.dma_start(out=b1_sb, in_=b1[:, :])
        b2_sb = singles.tile([BLK, 1], F32)
        nc.sync.dma_start(out=b2_sb, in_=b2[:, :])
        bc_sb = singles.tile([BLK, nblk], F32)
        nc.sync.dma_start(out=bc_sb, in_=bcols[:, :])
        # bf16 iota: values 0..127 exact; 2-byte operands enable DVE 2x mode
        iota_sb = singles.tile([BLK, G_LOC], BF16)
        nc.gpsimd.iota(
            out=iota_sb,
            pattern=[[1, G_LOC]],
            base=0,
            channel_multiplier=0,
            allow_small_or_imprecise_dtypes=True,
        )

        acc = acc_pool.tile([G_LOC, HIDDEN + 1], F32)

        def body():
            NCH = nsup * CPS
            chunks = [(s, q) for s in range(nsup) for q in range(CPS)]
            state = [dict() for _ in range(NCH)]
            supers = {}

            def dma_super(s):
                # xt first: stage_M consumes xt one pipeline stage before
                # stage_D consumes xa, so finish the xt transfer first
                # (issuing xa first serializes stage_M behind the 1MB xa
                # transfer - measured ~17us/pass slower).
                xt = xt_pool.tile([BLK, 2, SUP], FP8)
                nc.sync.dma_start(out=xt, in_=xT[s])
                xa = xa_pool.tile([BLK, NB, HIDDEN + 1], BF16)
                nc.sync.dma_start(out=xa, in_=xaug[s])
                supers[s] = {"xa": xa, "xt": xt}

            def stage_M(i):
                s, q = chunks[i]
                xt = supers[s]["xt"]
                hp = hp_pool.tile([H, CH], F32)
                nc.tensor.matmul(
                    hp,
                    lhsT=w1_sb,
                    rhs=xt[:, :, q * CH : (q + 1) * CH],
                    perf_mode=mybir.MatmulPerfMode.DoubleRow,
                    start=True,
                    stop=True,
                )
                tt = tt_pool.tile([H, CH], BF16)
                nc.scalar.activation(
                    out=tt, in_=hp, func=mybir.ActivationFunctionType.Tanh, bias=b1_sb
                )
                state[i]["tt"] = tt

            def stage_S(i):
                s, q = chunks[i]
                tt = state[i]["tt"]
                sp = sp_pool.tile([BLK, NBPC], F32)
                for b in range(NBPC):
                    nc.tensor.matmul(
                        sp[:, b : b + 1],
                        lhsT=tt[:, b * BLK : (b + 1) * BLK],
                        rhs=w2_sb,
                        start=True,
                        stop=True,
                    )
                ee = e_pool.tile([BLK, NBPC], F32)
                nc.scalar.activation(
                    out=ee, in_=sp, func=mybir.ActivationFunctionType.Exp, bias=b2_sb
                )
                sts = []
                for b in range(NBPC):
                    st = st_pool.tile([BLK, G_LOC], BF16, tag="st")
                    j = (s * CPS + q) * NBPC + b
                    nc.vector.tensor_scalar(
                        out=st,
                        in0=iota_sb,
                        scalar1=bc_sb[:, j : j + 1],
                        scalar2=ee[:, b : b + 1],
                        op0=mybir.AluOpType.is_equal,
                        op1=mybir.AluOpType.mult,
                    )
                    sts.append(st)
                state[i]["sts"] = sts

            def stage_D(i):
                s, q = chunks[i]
                xa = supers[s]["xa"]
                sts = state[i]["sts"]
                for b in range(NBPC):
                    nc.tensor.matmul(
                        acc,
                        lhsT=sts[b],
                        rhs=xa[:, q * NBPC + b, :],
                        start=(i == 0 and b == 0),
                        stop=(i == NCH - 1 and b == NBPC - 1),
                    )

            dma_super(0)
            for i in range(NCH + 2):
                if i < NCH:
                    s, q = chunks[i]
                    if q == 0 and s + 1 < nsup:
                        dma_super(s + 1)
                    stage_M(i)
                if 0 <= i - 1 < NCH:
                    stage_S(i - 1)
                if 0 <= i - 2 < NCH:
                    stage_D(i - 2)

        if repeats == 1:
            body()
        else:
            with tc.For_i(0, repeats):
                body()

        denom = singles.tile([G_LOC, 1], F32)
        nc.vector.tensor_scalar_max(
            out=denom, in0=acc[:, HIDDEN : HIDDEN + 1], scalar1=1e-30
        )
        rdenom = singles.tile([G_LOC, 1], F32)
        nc.vector.reciprocal(out=rdenom, in_=denom)
        out_sb = singles.tile([G_LOC, HIDDEN], F32)
        nc.vector.tensor_scalar_mul(out=out_sb, in0=acc[:, 0:HIDDEN], scalar1=rdenom)
        nc.sync.dma_start(out=out[:, :], in_=out_sb)

    nc.finalize()
    return nc


def make_in_maps(x, batch, W1, b1, W2, b2):
    """Shard by graph (128 contiguous graphs per core), pad node counts to a
    common multiple of SUP, and lay out the per-core device arrays."""
    x = np.asarray(x, dtype=np.float32)
    batch = np.asarray(batch)
    bounds = np.searchsorted(batch, np.arange(0, NUM_GRAPHS + 1, G_LOC))
    n_loc_max = int(np.diff(bounds).max())
    n_pad = max(SUP, ((n_loc_max + SUP - 1) // SUP) * SUP)

    w1_q = np.asarray(W1, np.float32).astype(ml_dtypes.float8_e4m3)
    w2_bf = np.asarray(W2, np.float32).reshape(H, 1).astype(ml_dtypes.bfloat16)
    b1_f = np.asarray(b1, np.float32).reshape(H, 1)
    b2_f = np.full((BLK, 1), np.float32(np.asarray(b2).reshape(-1)[0]), np.float32)

    in_maps = []
    for c in range(N_CORES):
        s, e = int(bounds[c]), int(bounds[c + 1])
        nloc = e - s
        xs = x[s:e]
        nsup = n_pad // SUP
        xa = np.zeros((n_pad, HIDDEN + 1), ml_dtypes.bfloat16)
        xa[:nloc, :HIDDEN] = xs
        xa[:nloc, HIDDEN] = 1.0
        # [s*SUP + b*BLK + p, f] -> [s, p, b, f]
        xa = np.ascontiguousarray(
            xa.reshape(nsup, NB, BLK, HIDDEN + 1).transpose(0, 2, 1, 3)
        )
        # [s, p, j, n] = x[s*SUP + n, BLK*j + p]
        xT = np.zeros((HIDDEN, n_pad), ml_dtypes.float8_e4m3)
        xT[:, :nloc] = xs.T.astype(ml_dtypes.float8_e4m3)
        xT = np.ascontiguousarray(xT.reshape(2, BLK, nsup, SUP).transpose(2, 1, 0, 3))
        bl = np.full((n_pad,), -1.0, np.float32)
        bl[:nloc] = batch[s:e].astype(np.float32) - np.float32(c * G_LOC)
        bcols = np.ascontiguousarray(bl.reshape(n_pad // BLK, BLK).T)
        in_maps.append(
            {
                "xaug": xa,
                "xT": xT,
                "bcols": bcols,
                "w1": w1_q,
                "w2": w2_bf,
                "b1": b1_f,
                "b2": b2_f,
            }
        )
    return in_maps, n_pad


def kernel(x, batch, W1, b1, W2, b2):
    from concourse.bass_utils import run_bass_kernel_spmd

    in_maps, n_pad = make_in_maps(x, batch, W1, b1, W2, b2)
    nc = _PROGRAM_CACHE.get(n_pad)
    if nc is None:
        nc = build_program(n_pad)
        _PROGRAM_CACHE[n_pad] = nc
    res = run_bass_kernel_spmd(nc, in_maps, list(range(N_CORES)))
    return np.concatenate([res.results[c]["out"] for c in range(N_CORES)], axis=0)


# revision 6
# speedup vs baseline: 1.1946x; 1.0886x over previous
"""AttentionPooling (segment softmax + weighted segment sum) on 8 trn2 cores.

Math (per graph g): out[g] = sum_n softmax_g(s)_n * x[n] over nodes n with
batch[n] == g, where s = tanh(x @ W1 + b1) @ W2 + b2.

Design (measured on hw, fastest of the variants tried):
  * batch is sorted -> shard by graph (128 contiguous graphs per core): pure
    data parallel, no collectives; host gathers the 8 [128, 256] outputs.
  * |s| <= ||W2||_1 + |b2| ~ 11.3, so exp never overflows fp32 -> accumulate
    unnormalized exp(s)*[x|1] into one PSUM tile and divide once at the end.
  * Two HBM streams: xaug = [x|1] in bf16 with nodes on partitions (pooling
    rhs), and xT in fp8-e4m3 with features on partitions (attention-MLP rhs).
    fp8 on the scores path costs ~1.3e-2 rel err (gate 2e-2) and cuts both
    HBM traffic and PE work.
  * MLP layer 1 runs as a single fp8 DoubleRow matmul per 512-node chunk
    (both K-tiles in one pass, 0.5 cyc/row).
  * The segment-sum is a TensorE matmul with a weighted one-hot matrix
    st[n, g] = exp(s_n) * (batch[n] == g), 128 local graphs == PSUM
    partitions; the one-hot is built on DVE (is_equal+mult tensor_scalar;
    GPSIMD is ~10x slower than its cost model for this op - measured).
  * Software pipeline: per chunk i the stages M(i) = MLP matmuls + tanh,
    S(i-1) = score matmuls + exp + one-hot, D(i-2) = pooling accumulation
    run in one PE pass, so PE never waits on ScalarE/DVE results of the
    same chunk (cross-engine dependencies get a full chunk of slack).

The `repeats` parameter wraps the whole pass in a hardware For_i loop; each
rep redoes the accumulation from scratch (output = last pass, still correct)
- used by test.py to measure per-pass HW time by wall-clock slope.
"""

import sys

from contextlib import ExitStack

import numpy as np

for _p in ("/opt/trn_rl_repo",):
    if _p not in sys.path:
        sys.path.insert(0, _p)

import ml_dtypes

import concourse.bass as bass
import concourse.bacc as bacc
import concourse.tile as tile
from concourse import mybir

N_NODES = 500_000
HIDDEN = 256
NUM_GRAPHS = 1024
N_CORES = 8
G_LOC = NUM_GRAPHS // N_CORES  # 128 graphs per core == PSUM partition dim
H = HIDDEN // 2  # 128 hidden units in the attention MLP
BLK = 128  # nodes per block (pool matmul contraction tile)
NBPC = 4  # blocks per chunk
CH = BLK * NBPC  # 512 nodes per compute chunk (one PSUM bank at fp32)
CPS = 4  # chunks per DMA super-chunk
SUP = CH * CPS  # 2048 nodes per DMA super-chunk
NB = NBPC * CPS  # 16 blocks per super-chunk
BF16 = mybir.dt.bfloat16
FP8 = mybir.dt.float8e4
F32 = mybir.dt.float32

_PROGRAM_CACHE: dict[int, bass.Bass] = {}


def build_program(n_pad: int, repeats: int = 1) -> bass.Bass:
    assert n_pad % SUP == 0
    nblk = n_pad // BLK
    nsup = n_pad // SUP

    nc = bacc.Bacc("TRN2")
    # host-swizzled so each super-chunk DMA reads one contiguous run per
    # partition: xaug[s, p, b, f] = [x | 1.0][s*SUP + b*BLK + p, f]
    xaug = nc.dram_tensor(
        "xaug", [nsup, BLK, NB, HIDDEN + 1], BF16, kind="ExternalInput"
    )
    # xT[s, p, j, n] = x[s*SUP + n, BLK*j + p] in fp8-e4m3
    xT = nc.dram_tensor("xT", [nsup, BLK, 2, SUP], FP8, kind="ExternalInput")
    bcols = nc.dram_tensor("bcols", [BLK, nblk], F32, kind="ExternalInput")
    w1 = nc.dram_tensor("w1", [HIDDEN, H], FP8, kind="ExternalInput")
    w2 = nc.dram_tensor("w2", [H, 1], BF16, kind="ExternalInput")
    b1 = nc.dram_tensor("b1", [H, 1], F32, kind="ExternalInput")
    b2 = nc.dram_tensor("b2", [BLK, 1], F32, kind="ExternalInput")
    out = nc.dram_tensor("out", [G_LOC, HIDDEN], F32, kind="ExternalOutput")

    with tile.TileContext(nc) as tc, ExitStack() as ctx:
        singles = ctx.enter_context(tc.tile_pool(name="singles", bufs=1))
        xa_pool = ctx.enter_context(tc.tile_pool(name="xa", bufs=4))
        xt_pool = ctx.enter_context(tc.tile_pool(name="xt", bufs=4))
        tt_pool = ctx.enter_context(tc.tile_pool(name="tt", bufs=4))
        st_pool = ctx.enter_context(tc.tile_pool(name="st", bufs=12))
        e_pool = ctx.enter_context(tc.tile_pool(name="e", bufs=4))
        hp_pool = ctx.enter_context(tc.tile_pool(name="hp", bufs=2, space="PSUM"))
        sp_pool = ctx.enter_context(tc.tile_pool(name="sp", bufs=2, space="PSUM"))
        acc_pool = ctx.enter_context(tc.tile_pool(name="acc", bufs=1, space="PSUM"))

        w1_sb = singles.tile([BLK, 2, H], FP8)
        nc.sync.dma_start(out=w1_sb[:, 0, :], in_=w1[0:BLK, :])
        nc.sync.dma_start(out=w1_sb[:, 1, :], in_=w1[BLK : 2 * BLK, :])
        w2_sb = singles.tile([H, 1], BF16)
        nc.sync.dma_start(out=w2_sb, in_=w2[:, :])
        b1_sb = singles.tile([H, 1], F32)
        nc.sync.dma_start(out=b1_sb, in_=b1[:, :])
        b2_sb = singles.tile([BLK, 1], F32)
        nc.sync.dma_start(out=b2_sb, in_=b2[:, :])
        bc_sb = singles.tile([BLK, nblk], F32)
        nc.sync.dma_start(out=bc_sb, in_=bcols[:, :])
        # bf16 iota: values 0..127 exact; 2-byte operands enable DVE 2x mode
        iota_sb = singles.tile([BLK, G_LOC], BF16)
        nc.gpsimd.iota(
            out=iota_sb,
            pattern=[[1, G_LOC]],
            base=0,
            channel_multiplier=0,
            allow_small_or_imprecise_dtypes=True,
        )

        acc = acc_pool.tile([G_LOC, HIDDEN + 1], F32)

        def body():
            NCH = nsup * CPS
            chunks = [(s, q) for s in range(nsup) for q in range(CPS)]
            state = [dict() for _ in range(NCH)]
            supers = {}

            # xt runs 2 supers ahead, xa 1 ahead: stage_M consumes xt one
            # pipeline stage before stage_D consumes xa, and the deeper xt
            # prefetch fills the DMA idle time between supers (issuing xa
            # first / shallow xt prefetch measured 10-20us/pass slower).
            def dma_xt(s):
                xt = xt_pool.tile([BLK, 2, SUP], FP8)
                nc.sync.dma_start(out=xt, in_=xT[s])
                supers.setdefault(s, {})["xt"] = xt

            def dma_xa(s):
                xa = xa_pool.tile([BLK, NB, HIDDEN + 1], BF16)
                nc.sync.dma_start(out=xa, in_=xaug[s])
                supers.setdefault(s, {})["xa"] = xa

            def stage_M(i):
                s, q = chunks[i]
                xt = supers[s]["xt"]
                hp = hp_pool.tile([H, CH], F32)
                nc.tensor.matmul(
                    hp,
                    lhsT=w1_sb,
                    rhs=xt[:, :, q * CH : (q + 1) * CH],
                    perf_mode=mybir.MatmulPerfMode.DoubleRow,
                    start=True,
                    stop=True,
                )
                tt = tt_pool.tile([H, CH], BF16)
                nc.scalar.activation(
                    out=tt, in_=hp, func=mybir.ActivationFunctionType.Tanh, bias=b1_sb
                )
                state[i]["tt"] = tt

            def stage_S(i):
                s, q = chunks[i]
                tt = state[i]["tt"]
                sp = sp_pool.tile([BLK, NBPC], F32)
                for b in range(NBPC):
                    nc.tensor.matmul(
                        sp[:, b : b + 1],
                        lhsT=tt[:, b * BLK : (b + 1) * BLK],
                        rhs=w2_sb,
                        start=True,
                        stop=True,
                    )
                ee = e_pool.tile([BLK, NBPC], F32)
                nc.scalar.activation(
                    out=ee, in_=sp, func=mybir.ActivationFunctionType.Exp, bias=b2_sb
                )
                sts = []
                for b in range(NBPC):
                    st = st_pool.tile([BLK, G_LOC], BF16, tag="st")
                    j = (s * CPS + q) * NBPC + b
                    nc.vector.tensor_scalar(
                        out=st,
                        in0=iota_sb,
                        scalar1=bc_sb[:, j : j + 1],
                        scalar2=ee[:, b : b + 1],
                        op0=mybir.AluOpType.is_equal,
                        op1=mybir.AluOpType.mult,
                    )
                    sts.append(st)
                state[i]["sts"] = sts

            def stage_D(i):
                s, q = chunks[i]
                xa = supers[s]["xa"]
                sts = state[i]["sts"]
                for b in range(NBPC):
                    nc.tensor.matmul(
                        acc,
                        lhsT=sts[b],
                        rhs=xa[:, q * NBPC + b, :],
                        start=(i == 0 and b == 0),
                        stop=(i == NCH - 1 and b == NBPC - 1),
                    )

            dma_xt(0)
            if nsup > 1:
                dma_xt(1)
            dma_xa(0)
            for i in range(NCH + 2):
                if i < NCH:
                    s, q = chunks[i]
                    if q == 0:
                        if s + 2 < nsup:
                            dma_xt(s + 2)
                        if s + 1 < nsup:
                            dma_xa(s + 1)
                    stage_M(i)
                if 0 <= i - 1 < NCH:
                    stage_S(i - 1)
                if 0 <= i - 2 < NCH:
                    stage_D(i - 2)

        if repeats == 1:
            body()
        else:
            with tc.For_i(0, repeats):
                body()

        denom = singles.tile([G_LOC, 1], F32)
        nc.vector.tensor_scalar_max(
            out=denom, in0=acc[:, HIDDEN : HIDDEN + 1], scalar1=1e-30
        )
        rdenom = singles.tile([G_LOC, 1], F32)
        nc.vector.reciprocal(out=rdenom, in_=denom)
        out_sb = singles.tile([G_LOC, HIDDEN], F32)
        nc.vector.tensor_scalar_mul(out=out_sb, in0=acc[:, 0:HIDDEN], scalar1=rdenom)
        nc.sync.dma_start(out=out[:, :], in_=out_sb)

    nc.finalize()
    return nc


def make_in_maps(x, batch, W1, b1, W2, b2):
    """Shard by graph (128 contiguous graphs per core), pad node counts to a
    common multiple of SUP, and lay out the per-core device arrays."""
    x = np.asarray(x, dtype=np.float32)
    batch = np.asarray(batch)
    bounds = np.searchsorted(batch, np.arange(0, NUM_GRAPHS + 1, G_LOC))
    n_loc_max = int(np.diff(bounds).max())
    n_pad = max(SUP, ((n_loc_max + SUP - 1) // SUP) * SUP)

    w1_q = np.asarray(W1, np.float32).astype(ml_dtypes.float8_e4m3)
    w2_bf = np.asarray(W2, np.float32).reshape(H, 1).astype(ml_dtypes.bfloat16)
    b1_f = np.asarray(b1, np.float32).reshape(H, 1)
    b2_f = np.full((BLK, 1), np.float32(np.asarray(b2).reshape(-1)[0]), np.float32)

    in_maps = []
    for c in range(N_CORES):
        s, e = int(bounds[c]), int(bounds[c + 1])
        nloc = e - s
        xs = x[s:e]
        nsup = n_pad // SUP
        xa = np.zeros((n_pad, HIDDEN + 1), ml_dtypes.bfloat16)
        xa[:nloc, :HIDDEN] = xs
        xa[:nloc, HIDDEN] = 1.0
        # [s*SUP + b*BLK + p, f] -> [s, p, b, f]
        xa = np.ascontiguousarray(
            xa.reshape(nsup, NB, BLK, HIDDEN + 1).transpose(0, 2, 1, 3)
        )
        # [s, p, j, n] = x[s*SUP + n, BLK*j + p]
        xT = np.zeros((HIDDEN, n_pad), ml_dtypes.float8_e4m3)
        xT[:, :nloc] = xs.T.astype(ml_dtypes.float8_e4m3)
        xT = np.ascontiguousarray(xT.reshape(2, BLK, nsup, SUP).transpose(2, 1, 0, 3))
        bl = np.full((n_pad,), -1.0, np.float32)
        bl[:nloc] = batch[s:e].astype(np.float32) - np.float32(c * G_LOC)
        bcols = np.ascontiguousarray(bl.reshape(n_pad // BLK, BLK).T)
        in_maps.append(
            {
                "xaug": xa,
                "xT": xT,
                "bcols": bcols,
                "w1": w1_q,
                "w2": w2_bf,
                "b1": b1_f,
                "b2": b2_f,
            }
        )
    return in_maps, n_pad


def kernel(x, batch, W1, b1, W2, b2):
    from concourse.bass_utils import run_bass_kernel_spmd

    in_maps, n_pad = make_in_maps(x, batch, W1, b1, W2, b2)
    nc = _PROGRAM_CACHE.get(n_pad)
    if nc is None:
        nc = build_program(n_pad)
        _PROGRAM_CACHE[n_pad] = nc
    res = run_bass_kernel_spmd(nc, in_maps, list(range(N_CORES)))
    return np.concatenate([res.results[c]["out"] for c in range(N_CORES)], axis=0)
